# revision 34
# baseline (speedup 1.0000x reference)
"""Trainium2 Bass kernel for nn_CostFn_18562848653837.

reference(x, cond, time) only reads x[b, j, 6+k] for j in [0,26), k in [0,6)
(~2.6 MB of the 436 MB input; cond/time are unused) and computes, per point,
the reflected mass 1 / (u^T J M^{-1} J^T u) with u = e_x, which reduces via
Sherman-Morrison (M = 2I + 0.5 c c^T) to pure functions of sin^2(cq) and
sin(2*cq), cq = cumsum(q):

    Q1 = sum_k L_k^2 sin^2(cq_k)
    Q3 = sum_k sin^2(cq_k)
    P2 = sum_k L_k sin(2 cq_k)
    TC = 2.5 - 0.25*Q3
    cost = TC / (0.5*Q1*TC - P2^2/32)

Both sin^2(th) and sin(2 th) are invariant under th -> th - k*pi, so the
host ships m = cq/pi - rne(cq/pi) in [-0.5, 0.5] as bf16 (phase error
~2^-10 pi; measured end-to-end rel err ~1.5e-5 vs the 2e-2 gate), halving
DMA bytes. The device does all the nonlinear math: both Sins on ACT (the
radians conversion rides the ACT input scale; |pi*m| <= pi/2 and
|2pi(1-2^-23)m| < pi keep the table domain satisfied), squares + the three
6-plane accumulations, and the final G -> D -> reciprocal -> cost chain
with the column sum fused into the last scalar_tensor_tensor via
accum_out.

Schedule (CoreSim 6725ns, fully saturated: 1483 entry + 1780 ACT + 945
tail + 2217 out-DMA + ~300 fused exit): the Sin table-set load (1283ns)
is pre-placed INSIDE the Bacc init barrier - after ACT's drain, before its
release-wait - so it runs during the barrier exchange and the first Sin
starts at 1483; ACT then runs 4 saturated Sin chunks (SM first - its
SQ->{Q3,TC,Q1,G} tail is long; SF after - its P2 tail is short). Pool
(which HW codegen limits to tensor_scalar/tensor_tensor; stt is DVE-only)
does SQ_k, the Q3 adds, TC, H/G, and the P2a(planes 0-2) block,
order-nudged so P2a lands right as SF345 finishes; DVE runs the Q1 stt
chain, then the critical P2(3-5) stt chain into ONE custom 8-stage op
(registered at runtime with computed uops_sha) that fuses
D = G - P2^2/32 with a seeded 1-Newton reciprocal, then the final stt
whose accum_out fuses the column sum. Intermediates are bf16 (f32 only
for TC/G/R/cost, guarding the small-denominator points); the P2 and G
sides converge co-critically.

Sharding: pure data parallel over batch - core i gets batches
[512*i, 512*(i+1)), i.e. 13312 points as (128, 104) per q-component.
Each core emits a (128,1) f32 partial; host adds the 8*128 values.
"""

import numpy as np

_P, _W, _K = 128, 104, 6
_NCORES = 8
_B, _H, _T = 4096, 1024, 26
_BPC = _B // _NCORES  # batches per core
_NCHUNK = 2           # DMA chunks, 3 planes each
_CW = 3 * _W          # chunk width (312)

_CACHE = {}


def _get_d_op():
    """Register (once) a fused custom DVE op computing, in one 8-stage pass,
    x = in0 - s0*in1^2  (the denominator D = G - P2^2/32) and then an
    approximate 1/x: BITWISE_NOT exponent-flip seed (x*bitcast(~x) lands in
    [-4.5,-4] for any |x|), Chebyshev scale s1, one Newton step via imm2.
    One fewer NR step than reciprocal_approx_fast (~0.17% worst-case,
    one-sided); measured end-to-end rel err 1.5e-5 vs the 2e-2 gate."""
    if "d_op" in _CACHE:
        return _CACHE["d_op"]
    import concourse.dve_ops as dve_ops
    from concourse.dve_ops import DveOp, OPS, CUSTOM_DVE_SPECS, _SUB_OPCODE_FOR_NAME
    from concourse.dve_spec import Spec, Src0, Src1, C0, C1, C2, sq, lower, AluOp
    from concourse.dve_spec import Bin
    from concourse.dve_uop import DveOpSpec

    name = "SUB_SQ_RECIP_ANT"
    if name not in _SUB_OPCODE_FOR_NAME:
        _x = Src0 - sq(Src1) * C0
        _n = Bin(AluOp.BITWISE_NOT, _x, _x)
        _y0 = _n * C1

        def _ref(in0, in1, s0, s1, imm2):
            x = np.ascontiguousarray(
                in0.astype(np.float32) - in1.astype(np.float32) ** 2 * np.float32(s0)
            ).astype(np.float32)
            n = (~x.view(np.int32)).view(np.float32)
            y0 = (n * np.float32(s1)).astype(np.float32)
            return (y0 * (np.float32(imm2) - x * y0)).astype(np.float32)

        spec = Spec(
            body=_y0 * (C2 - _x * _y0),
            reference=_ref,
        )
        row = max(_SUB_OPCODE_FOR_NAME.values()) + 1
        assert row < 0x20
        shas = {}
        for ver in ("v3", "v4"):
            shas[ver] = DveOpSpec(
                name=name, opcode=row, uops=lower(spec, ver=ver), rd1_en=True
            ).sha(ver)
        op = DveOp(name, spec, subdim=False, uops_sha=shas)
        OPS.append(op)
        CUSTOM_DVE_SPECS[name] = spec
        _SUB_OPCODE_FOR_NAME[name] = row
        _CACHE["d_op"] = op
    else:
        _CACHE["d_op"] = next(o for o in dve_ops.OPS if o.name == name)
    return _CACHE["d_op"]


def _get_nc():
    if "nc" in _CACHE:
        return _CACHE["nc"]

    import concourse.tile as tile
    import concourse.mybir as mybir
    from concourse import bacc

    PI32 = float(np.float32(np.pi))
    # One-ulp-shaded 2*pi: |m| <= 0.5 exactly, so the Sin input
    # |SCALE2*m| <= pi*(1-2^-23) stays strictly inside the table domain.
    SCALE2 = float(np.float32(2.0 * np.pi * (1.0 - 2.0**-23)))
    L = [float(np.float32(v)) for v in np.arange(1, 7) * 0.1 + 0.3]

    f32 = mybir.dt.float32
    bf16 = mybir.dt.bfloat16
    OP = mybir.AluOpType
    ACT = mybir.ActivationFunctionType

    # Pre-place the Sin table-set load (act_func_set_id 9 = trig set, the
    # id the compile pass itself picks for this kernel) INSIDE the Bacc
    # init barrier: right after ACT's drain (which fires its gather sem at
    # t~100) and before its release-wait. The 1283ns load then runs during
    # the barrier exchange instead of after it, so the first Sin can start
    # ~200ns earlier. Correctness is unaffected: the load has no operands,
    # and every data-consuming ACT op still carries its DMA sem wait.
    import concourse.bass as cbass

    _ACT_SET_ID = 9
    orig_barrier = cbass.Bass.all_engine_barrier

    def _barrier_with_act_load(self, *, sem_only: bool = False):
        if sem_only:
            return orig_barrier(self, sem_only=True)
        for inst in self._multi_engine_barrier_insts(list(self.engines)):
            self.engines[inst.engine].add_instruction(inst)
            if (
                isinstance(inst, mybir.InstDrain)
                and inst.engine == mybir.EngineType.Activation
            ):
                load = mybir.InstLoadActFuncSet(
                    name=self.get_next_instruction_name(),
                    ins=[],
                    outs=[],
                    act_func_set_id=_ACT_SET_ID,
                )
                load.engine = mybir.EngineType.Activation
                self.engines[mybir.EngineType.Activation].add_instruction(load)

    cbass.Bass.all_engine_barrier = _barrier_with_act_load
    try:
        nc = bacc.Bacc(
            "TRN2", target_bir_lowering=False, debug=False, num_devices=_NCORES,
            disable_frame_to_traceback=True,
        )
    finally:
        cbass.Bass.all_engine_barrier = orig_barrier
    q_dram = nc.dram_tensor("q", [_NCHUNK, _P, _CW], bf16, kind="ExternalInput")
    out_dram = nc.dram_tensor("out", [_P, 1], f32, kind="ExternalOutput")

    d_op = _get_d_op()

    # Fused exit: the stock epilogue is [SP drain(wait out-DMA), all-engine
    # barrier, sem-range clear, all-engine barrier] (~600ns). Semantically
    # the clear only needs (a) every engine past its last sem use - true
    # once the leader has consumed all gather increments - and (b) no engine
    # resuming before it - guaranteed by holding the leader's release. So
    # emit the clear inside the first barrier's leader critical section
    # (Pool is the leader: first in nc.engines; the clear ops are
    # gpsimd-emitted anyway) and drop the second barrier round entirely.
    from concourse.vector_clock import ScopedClock

    orig_dab = tile.TileContext._drain_and_barrier

    def _fused_drain_and_barrier(self, tick_clock, wait_clock):
        ncc = self.nc
        engines = list(ncc.engines)
        if engines[0] != mybir.EngineType.Pool:
            return orig_dab(self, tick_clock, wait_clock)
        insts = ncc._multi_engine_barrier_insts(engines)
        # Attach the tile-clock waits (incl. the out-DMA completion)
        # directly to the barrier's own SP follower drain instead of
        # emitting a separate sync.drain() before it.
        sp_drain = next(
            inst for inst in insts
            if isinstance(inst, mybir.InstDrain)
            and inst.engine == mybir.EngineType.SP
        )
        wait_clock.add_sem_waits(
            sp_drain, ScopedClock({None: tick_clock.global_clock})
        )
        assert self.sems is not None
        popped = ncc._tile_sem_poison_stack.pop()
        assert popped is self._sem_poison
        last_pool = max(
            i for i, inst in enumerate(insts)
            if inst.engine == mybir.EngineType.Pool
        )
        for i, inst in enumerate(insts):
            if i == last_pool:  # leader release: clear just before it
                ncc.clear_and_free_semaphores(
                    list(self.sems.allocated().values())
                )
            ncc.engines[inst.engine].add_instruction(inst)

    tile.TileContext._drain_and_barrier = _fused_drain_and_barrier
    try:
        with (
            tile.TileContext(nc) as tc,
            tc.tile_pool(name="pool", bufs=1) as pool,
        ):
            _build_body(nc, tc, pool, tile, mybir, q_dram, out_dram, d_op,
                        PI32, SCALE2, L, f32, bf16, OP, ACT)
    finally:
        tile.TileContext._drain_and_barrier = orig_dab

    nc.compile()
    _CACHE["nc"] = nc
    return nc


def _build_body(nc, tc, pool, tile, mybir, q_dram, out_dram, d_op,
                PI32, SCALE2, L, f32, bf16, OP, ACT):
    if True:
        # Dep-free dummy Sin on the const-1.0 AP keeps the pre-placed
        # table load adopted by insert_act_table_loads (no second load).
        one_ap = nc.const_aps.aps[(f32, 1.0)]
        WARM = pool.tile([_P, 1], f32)
        nc.scalar.activation(WARM[:], one_ap[:_P], ACT.Sin)

        # Two input chunks (planes 0-2 / 3-5) on the two DMA-capable
        # sequencers so both issue in parallel.
        Qc = []
        for c in range(_NCHUNK):
            qc = pool.tile([_P, _CW], bf16, tag=f"q{c}")
            eng = nc.sync if c == 0 else nc.gpsimd
            eng.dma_start(qc[:], q_dram[c])
            Qc.append(qc)

        # ACT: SM chunks first (their downstream SQ->{Q3,TC,Q1,G} chain is
        # the long one), SF chunks after (their P2 tail is shorter).
        SM = pool.tile([_P, _K * _W], bf16)
        SF = pool.tile([_P, _K * _W], bf16)
        for c in range(_NCHUNK):
            sl = slice(c * _CW, (c + 1) * _CW)
            nc.scalar.activation(SM[:, sl], Qc[c][:], ACT.Sin, scale=PI32)
        for c in range(_NCHUNK):
            sl = slice(c * _CW, (c + 1) * _CW)
            nc.scalar.activation(SF[:, sl], Qc[c][:], ACT.Sin, scale=SCALE2)

        # Pool (tensor_scalar/tensor_tensor only - stt is DVE-only on HW):
        # squares, Q3 accumulation, TC; DVE: Q1 stt chain.
        SQ = pool.tile([_P, _K * _W], bf16)
        Q3 = pool.tile([_P, _W], bf16)
        Q1 = pool.tile([_P, _W], bf16)
        q3_tail = []  # Pool ops that may yield to the P2a block
        for k in range(_K):
            sl = slice(k * _W, (k + 1) * _W)
            sq_i = nc.gpsimd.tensor_mul(SQ[:, sl], SM[:, sl], SM[:, sl])
            if k == 1:
                q3_i = nc.gpsimd.tensor_add(Q3[:], SQ[:, 0:_W], SQ[:, _W : 2 * _W])
                q3_tail.append(q3_i)
            elif k > 1:
                q3_i = nc.gpsimd.tensor_add(Q3[:], Q3[:], SQ[:, sl])
                q3_tail.append(q3_i)
            del sq_i
            if k == 0:
                nc.vector.tensor_scalar_mul(Q1[:], SQ[:, sl], L[0] * L[0])
            else:
                nc.vector.scalar_tensor_tensor(
                    Q1[:], SQ[:, sl], L[k] * L[k], Q1[:], OP.mult, OP.add
                )
        TC = pool.tile([_P, _W], f32)
        nc.gpsimd.tensor_scalar(TC[:], Q3[:], -0.25, 2.5, OP.mult, OP.add)

        # G = 0.5*Q1*TC as two Pool-legal ops (fills Pool slack; keeps DVE
        # free for the P2 tail chain).
        H = pool.tile([_P, _W], f32)
        nc.gpsimd.tensor_scalar_mul(H[:], Q1[:], 0.5)
        G = pool.tile([_P, _W], f32)
        nc.gpsimd.tensor_mul(G[:], H[:], TC[:])

        # P2: planes 0-2 accumulate on Pool right off SF012; planes 3-5
        # continue as a DVE stt chain straight into the D/R/FIN tail.
        WS = pool.tile([_P, 3 * _W], bf16)
        p2a_block = []
        for k in range(3):
            sl = slice(k * _W, (k + 1) * _W)
            p2a_block.append(nc.gpsimd.tensor_scalar_mul(WS[:, sl], SF[:, sl], L[k]))
        PA = pool.tile([_P, _W], bf16)
        p2a_block.append(nc.gpsimd.tensor_add(PA[:], WS[:, 0:_W], WS[:, _W : 2 * _W]))
        P2a = pool.tile([_P, _W], bf16)
        p2a_last = nc.gpsimd.tensor_add(P2a[:], PA[:], WS[:, 2 * _W : 3 * _W])
        p2a_block.append(p2a_last)
        # order-only edges: once SF012 lands, the P2a block takes priority
        # on Pool over the remaining Q3/SQ tail (whose consumers G/TC are
        # only needed ~700ns later)
        for late in q3_tail[-2:]:
            for early in p2a_block[-2:]:
                tile.add_dep_helper(
                    late.ins, early.ins, sync=False,
                    reason="P2a block before Q3 tail",
                )
        P2 = pool.tile([_P, _W], bf16)
        nc.vector.scalar_tensor_tensor(
            P2[:], SF[:, 3 * _W : 4 * _W], L[3], P2a[:], OP.mult, OP.add
        )
        nc.vector.scalar_tensor_tensor(
            P2[:], SF[:, 4 * _W : 5 * _W], L[4], P2[:], OP.mult, OP.add
        )
        nc.vector.scalar_tensor_tensor(
            P2[:], SF[:, 5 * _W : 6 * _W], L[5], P2[:], OP.mult, OP.add
        )

        # DVE tail: D = G - P2^2/32 (fused custom op); R ~= 1/D (1-pass
        # approx, ~51 ULP); cost = R*TC with the column sum fused via
        # accum_out.
        R = pool.tile([_P, _W], f32)
        nc.vector._custom_dve(
            d_op, out=R[:], in0=G[:], in1=P2[:],
            s0=1.0 / 32.0, s1=-0.23549792, imm2=2.0017324,
        )
        COST = pool.tile([_P, _W], f32)
        CS = pool.tile([_P, 1], f32)
        nc.vector.scalar_tensor_tensor(
            COST[:], R[:], 1.0, TC[:], OP.mult, OP.mult, accum_out=CS[:]
        )
        nc.sync.dma_start(out_dram[:], CS[:])


def _shard(x):
    """Host prep: slice, cumsum over joints, /pi, RNE range-reduce to
    [-0.5,0.5], bf16, and lay out per core as [2 chunks, 128, 3*104]
    (chunk c = planes 3c..3c+2, k-major within the chunk)."""
    import ml_dtypes

    q = np.asarray(x[:, :_T, 6 : 6 + _K], dtype=np.float32)
    g = np.cumsum(q, axis=-1, dtype=np.float32) * np.float32(1.0 / np.pi)
    m = (g - np.rint(g)).astype(np.float32)
    # (B, T, K) -> (cores, K, P, W) point-major layout per plane
    planes = (
        m.reshape(_NCORES, _BPC * _T, _K)
        .transpose(0, 2, 1)
        .reshape(_NCORES, _K, _P, _W)
    )
    # group planes into chunks of 3: (cores, 2, 128, 312)
    chunks = (
        planes.reshape(_NCORES, _NCHUNK, 3, _P, _W)
        .transpose(0, 1, 3, 2, 4)
        .reshape(_NCORES, _NCHUNK, _P, _CW)
    )
    return np.ascontiguousarray(chunks.astype(ml_dtypes.bfloat16))


def _get_runner():
    """Build the jitted 8-core shard_map executable once (mirrors
    bass2jax.run_bass_via_pjrt's multi-core path) so repeat kernel() calls
    skip retracing/recompiling."""
    if "run" in _CACHE:
        return _CACHE["run"]
    import jax
    from jax.sharding import Mesh, PartitionSpec
    from jax.experimental.shard_map import shard_map
    from concourse import bass2jax

    nc = _get_nc()
    bass2jax.install_neuronx_cc_hook()
    assert nc.dbg_addr is None
    pid_name = nc.partition_id_tensor.name if nc.partition_id_tensor else None
    in_names = ("q", "out") + ((pid_name,) if pid_name else ())

    out_aval = jax.core.ShapedArray((_P, 1), np.float32)

    def _body(q, out_zero):
        operands = [q, out_zero]
        if pid_name is not None:
            operands.append(bass2jax.partition_id_tensor())
        (out,) = bass2jax._bass_exec_p.bind(
            *operands,
            out_avals=(out_aval,),
            in_names=in_names,
            out_names=("out",),
            lowering_input_output_aliases=(),
            sim_require_finite=True,
            sim_require_nnan=True,
            nc=nc,
        )
        return (out,)

    devices = jax.devices()[:_NCORES]
    mesh = Mesh(np.asarray(devices), ("core",))
    sharded = jax.jit(
        shard_map(
            _body,
            mesh=mesh,
            in_specs=(PartitionSpec("core"),) * 2,
            out_specs=(PartitionSpec("core"),),
            check_rep=False,
        ),
        donate_argnums=(1,),
        keep_unused=True,
    )

    def run(planes):
        concat_q = planes.reshape(_NCORES * _NCHUNK, _P, _CW)
        zeros = np.zeros((_NCORES * _P, 1), np.float32)
        (out,) = sharded(concat_q, zeros)
        return np.asarray(out)  # (8*128, 1)

    _CACHE["run"] = run
    return run


def _run_library(planes):
    from concourse.bass_utils import run_bass_kernel_spmd

    res = run_bass_kernel_spmd(
        _get_nc(),
        [{"q": planes[i]} for i in range(_NCORES)],
        list(range(_NCORES)),
    )
    return np.stack([r["out"][:, 0] for r in res.results]).astype(np.float32)


def _run_subprocess(planes):
    """Last resort: the accelerator occasionally reports
    NRT_EXEC_UNIT_UNRECOVERABLE; a fresh process reliably recovers it."""
    import os
    import subprocess
    import sys
    import tempfile

    d = tempfile.mkdtemp()
    inp = os.path.join(d, "planes.npy")
    out = os.path.join(d, "out.npy")
    # np.save round-trips bf16 as raw V2 bytes; ship as uint16 and view back
    np.save(inp, planes.view(np.uint16))
    here = os.path.dirname(os.path.abspath(__file__))
    script = (
        "import sys, numpy as np, ml_dtypes\n"
        f"sys.path.insert(0, {here!r})\n"
        "import kernel as K\n"
        f"planes = np.load({inp!r}).view(ml_dtypes.bfloat16)\n"
        "out = K._get_runner()(planes)\n"
        f"np.save({out!r}, out)\n"
    )
    err = None
    for _ in range(2):
        try:
            subprocess.run(
                [sys.executable, "-c", script], check=True, timeout=900,
                stdout=subprocess.DEVNULL, stderr=subprocess.DEVNULL,
            )
            return np.load(out).astype(np.float32)
        except Exception as e:  # retry once; device usually recovers
            err = e
    raise err


def kernel(x, cond, time):
    x = np.asarray(x)
    planes = _shard(x)
    try:
        partials = _get_runner()(planes).astype(np.float32)
    except Exception:
        try:
            # library SPMD runner (covers fast-path/jax API drift)
            partials = _run_library(planes)
        except Exception:
            # fresh process recovers a wedged accelerator
            partials = _run_subprocess(planes)
    return np.float32(partials.sum(dtype=np.float32))
